# revision 8
# baseline (speedup 1.0000x reference)
"""Cosine-similarity loss kernel for Trainium2 (8 NeuronCores, SPMD).

loss = -sum_n dot(s_n, im_n) / (||s_n|| * ||im_n||)   for s, im in R^{65536 x 512}

Strategy (memory-bound; measured 102.4us exec vs 84us pure-stream floor):
  - Shard the 65536 rows across 8 cores (8192 rows each, 32 MB/core streamed).
  - One HWDGE ring (nc.sync) streams both tensors; all 16 SDMA engines run
    back-to-back (~400 GB/s/core, per-engine line rate) for ~84us.
  - Stream 128-row slices; per slice three fused one-pass reductions:
      dot = sum_d s*im  -> VectorE scalar_tensor_tensor, accum_out
      ss  = sum_d s*s   -> ScalarE activation(Square, accum_out)
      ii  = sum_d im*im -> DVE/ACT split per ii_sched (both ~89% busy)
  - Ship the raw per-slice stats as ONE contiguous [128, 3, slices] DMA
    (768B per partition line -> full-rate descriptors; a [128,1] out is 4B
    per partition = 128 RMW descriptors whose completion sem lags ~8us) and
    finish -sum(dot/sqrt(ss*ii)) on the host in f64.
  - Stats live in SEPARATE tiles (one shared tile adds cross-engine hazard
    stalls, ~15% compute slowdown) and are gathered with 3 DVE copies at
    the end.  f16-cast SWDGE streaming and gpsimd offload both measured
    slower (casting SDMA path is slower per descriptor; Pool cannot do
    free-dim reductions).  Remaining tail is ~7.3us of fixed NRT postamble.
"""

import numpy as np
from contextlib import ExitStack

import concourse.bacc as bacc
import concourse.bass as bass
import concourse.mybir as mybir
import concourse.tile as tile
from concourse.bass_utils import run_bass_kernel_spmd

N, D = 65536, 512
N_CORES = 8
ROWS = N // N_CORES          # 8192 rows per core
P = 128                      # SBUF partitions
F32 = mybir.dt.float32
F16 = mybir.dt.float16


def _build(
    rows=ROWS,
    # slices per DMA tile (1 slice = 128 rows = 256KB/tensor f32).  Small first
    # tiles start compute early; small last tiles shrink the post-DMA tail.
    seg_schedule=(1, 1, 2) + (4,) * 14 + (2, 1, 1),
    bufs=10,
    ii_sched="dddddddddddaaaaa",  # per-slice engine for ii: d=DVE a=ACT g=GPSIMD
    s_dma="sync",
    im_dma="sync",
    dt="f32",                # f32 | f16 (cast during DMA; forces SWDGE/gpsimd)
    stats_out=True,          # ship raw dot/ss/ii, host does -sum(dot/rsqrt)
    mapping="pj",
):
    slices = rows // P
    assert sum(seg_schedule) == slices
    sb_dt = F32 if dt == "f32" else F16
    if dt == "f16":
        s_dma = im_dma = "gpsimd"   # cast during DMA is SWDGE-only

    nc = bacc.Bacc(
        "TRN2", target_bir_lowering=False, debug=False, num_devices=N_CORES
    )
    s_d = nc.dram_tensor("s", [rows, D], F32, kind="ExternalInput").ap()
    im_d = nc.dram_tensor("im", [rows, D], F32, kind="ExternalInput").ap()
    if stats_out:
        out_d = nc.dram_tensor("out", [P, 3, slices], F32, kind="ExternalOutput").ap()
    else:
        out_d = nc.dram_tensor("out", [P, 1], F32, kind="ExternalOutput").ap()

    mult = mybir.AluOpType.mult

    with tile.TileContext(nc) as tc, ExitStack() as ctx:
        spool = ctx.enter_context(tc.tile_pool(name="spool", bufs=bufs))
        ipool = ctx.enter_context(tc.tile_pool(name="ipool", bufs=bufs))
        stats = ctx.enter_context(tc.tile_pool(name="stats", bufs=1))

        # Separate per-stat tiles: a single shared tile adds cross-engine
        # hazard stalls (~15% compute slowdown measured); gather at the end.
        dot_all = stats.tile([P, slices], F32)
        ss_all = stats.tile([P, slices], F32)
        ii_all = stats.tile([P, slices], F32)
        dve_scr = stats.tile([P, D], sb_dt)
        act_scr = stats.tile([P, D], sb_dt)
        gps_scr = (
            stats.tile([P, D], sb_dt, name="gps_scr") if "g" in ii_sched else None
        )

        def ii_op(engine, it, j, c):
            if engine == "a":
                nc.scalar.activation(
                    out=act_scr[:], in_=it[:, j, :],
                    func=mybir.ActivationFunctionType.Square,
                    accum_out=ii_all[:, c : c + 1],
                )
            else:
                eng = nc.vector if engine == "d" else nc.gpsimd
                scr = dve_scr if engine == "d" else gps_scr
                eng.scalar_tensor_tensor(
                    out=scr[:], in0=it[:, j, :], scalar=1.0, in1=it[:, j, :],
                    op0=mult, op1=mult,
                    accum_out=ii_all[:, c : c + 1],
                )

        c = 0
        r0 = 0
        pat = "(j p) d -> p j d" if mapping == "jp" else "(p j) d -> p j d"
        for seg in seg_schedule:
            nrows = seg * P
            s_seg = s_d[r0 : r0 + nrows, :].rearrange(pat, p=P, j=seg)
            im_seg = im_d[r0 : r0 + nrows, :].rearrange(pat, p=P, j=seg)
            r0 += nrows
            st = spool.tile([P, seg, D], sb_dt, name="st", tag="st")
            getattr(nc, s_dma).dma_start(st[:], s_seg)
            it = ipool.tile([P, seg, D], sb_dt, name="it", tag="it")
            getattr(nc, im_dma).dma_start(it[:], im_seg)
            for j in range(seg):
                nc.vector.scalar_tensor_tensor(
                    out=dve_scr[:], in0=st[:, j, :], scalar=1.0, in1=it[:, j, :],
                    op0=mult, op1=mult,
                    accum_out=dot_all[:, c : c + 1],
                )
                nc.scalar.activation(
                    out=act_scr[:], in_=st[:, j, :],
                    func=mybir.ActivationFunctionType.Square,
                    accum_out=ss_all[:, c : c + 1],
                )
                ii_op(ii_sched[c % len(ii_sched)], it, j, c)
                c += 1

        if stats_out:
            gather = stats.tile([P, 3, slices], F32, name="gather")
            nc.vector.tensor_copy(gather[:, 0, :], dot_all[:])
            nc.vector.tensor_copy(gather[:, 1, :], ss_all[:])
            nc.vector.tensor_copy(gather[:, 2, :], ii_all[:])
            nc.sync.dma_start(out_d, gather[:])
        else:
            # tail: loss_p = -sum_c dot_c * (ss_c*ii_c)^-1/2, via exp(-0.5*ln(x))
            prod = stats.tile([P, slices], F32)
            nc.vector.tensor_tensor(out=prod[:], in0=ss_all[:], in1=ii_all[:], op=mult)
            lnp = stats.tile([P, slices], F32)
            nc.scalar.activation(lnp[:], prod[:], mybir.ActivationFunctionType.Ln)
            rsq = stats.tile([P, slices], F32)
            nc.scalar.activation(
                rsq[:], lnp[:], mybir.ActivationFunctionType.Exp, scale=-0.5
            )
            fin_scr = stats.tile([P, slices], F32)
            loss_p = stats.tile([P, 1], F32)
            nc.vector.scalar_tensor_tensor(
                out=fin_scr[:], in0=dot_all[:], scalar=-1.0, in1=rsq[:],
                op0=mult, op1=mult,
                accum_out=loss_p[:],
            )
            nc.scalar.dma_start(out_d, loss_p[:])

    nc.compile()
    return nc


_compiled = None


def _get_nc():
    global _compiled
    if _compiled is None:
        _compiled = _build()
    return _compiled


def _run(s, im, nc=None, **kw):
    """Shard, run on 8 cores, return BassKernelResults."""
    s = np.ascontiguousarray(np.asarray(s, dtype=np.float32))
    im = np.ascontiguousarray(np.asarray(im, dtype=np.float32))
    assert s.shape == (N, D) and im.shape == (N, D)
    if nc is None:
        nc = _get_nc()
    in_maps = [
        {"s": s[c * ROWS : (c + 1) * ROWS], "im": im[c * ROWS : (c + 1) * ROWS]}
        for c in range(N_CORES)
    ]
    bkr = run_bass_kernel_spmd(nc, in_maps, core_ids=list(range(N_CORES)), **kw)
    return bkr


def _reduce(bkr):
    total = np.float64(0.0)
    for r in bkr.results:
        out = r["out"].astype(np.float64)
        if out.ndim == 3:          # raw stats [P, 3, slices]
            dot, ss, ii = out[:, 0, :], out[:, 1, :], out[:, 2, :]
            total += -(dot / np.sqrt(ss * ii)).sum()
        else:                      # on-device loss partials [P, 1]
            total += out.sum()
    return np.float32(total)


def kernel(s, im, temp=None, **_):
    return _reduce(_run(s, im))
